# revision 1
# baseline (speedup 1.0000x reference)
import sys

sys.path.insert(0, "/opt/trn_rl_repo")

import numpy as np
import ml_dtypes

import concourse.bass as bass
import concourse.mybir as mybir
import concourse.tile as tile
from concourse import bacc
from concourse.bass_utils import run_bass_kernel_spmd

BF16 = ml_dtypes.bfloat16
F32 = mybir.dt.float32
BF = mybir.dt.bfloat16
F32R = mybir.dt.float32r
ALU = mybir.AluOpType
ACTF = mybir.ActivationFunctionType
AX = mybir.AxisListType

NCORES = 8
B = 256
BL = B // NCORES          # 32 local batch
REC = 102400
RECL = REC // NCORES      # 12800 local output cols
NW = RECL // 512          # 25 output windows


def mkap(t, offset, dims):
    """Manual access pattern: dims = [[stride, count], ...] (partition dim first)."""
    return bass.AP(tensor=t.tensor if isinstance(t, bass.AP) else t, offset=offset, ap=dims)


def build_program():
    nc = bacc.Bacc(None, num_devices=NCORES)
    rg = [list(range(NCORES))]

    # ---- external params (per-core values supplied via in_maps) ----
    P = {}
    P["pat1h"] = nc.declare_dram_parameter("pat1h", [81, 4608], F32, isOutput=False)
    P["w1c"] = nc.declare_dram_parameter("w1c", [81, 256], F32, isOutput=False)
    P["b1c"] = nc.declare_dram_parameter("b1c", [256, 1], F32, isOutput=False)
    P["wp2"] = nc.declare_dram_parameter("wp2", [20736, 256], BF, isOutput=False)
    P["bp2"] = nc.declare_dram_parameter("bp2", [256, 1], F32, isOutput=False)
    P["w2s"] = nc.declare_dram_parameter("w2s", [128, 20480], BF, isOutput=False)
    P["m2"] = nc.declare_dram_parameter("m2", [128, BL], F32, isOutput=False)
    P["m4"] = nc.declare_dram_parameter("m4", [128, 4], F32, isOutput=False)
    P["ones32"] = nc.declare_dram_parameter("ones32", [BL, 1], F32, isOutput=False)
    P["onesrow"] = nc.declare_dram_parameter("onesrow", [1, 128], BF, isOutput=False)
    P["id32"] = nc.declare_dram_parameter("id32", [32, 32], F32, isOutput=False)
    P["w1t"] = nc.declare_dram_parameter("w1t", [160, 512], F32, isOutput=False)
    P["b1d"] = nc.declare_dram_parameter("b1d", [512, 1], F32, isOutput=False)
    P["w2t"] = nc.declare_dram_parameter("w2t", [512, 1024], F32, isOutput=False)
    P["b2d"] = nc.declare_dram_parameter("b2d", [1024, 1], F32, isOutput=False)
    P["w3t"] = nc.declare_dram_parameter("w3t", [1024, RECL], BF, isOutput=False)
    P["b3s"] = nc.declare_dram_parameter("b3s", [1, RECL], BF, isOutput=False)
    out_ext = nc.declare_dram_parameter("out", [B, RECL], F32, isOutput=True)

    with tile.TileContext(nc) as tc:
        _body(nc, tc, P, out_ext, rg)
    nc.compile()
    return nc


def _body(nc, tc, P, out_ext, rg):
    es = tc.tile_pool(name="const", bufs=1)
    const = es.__enter__()
    dram_cm = tc.tile_pool(name="dram", bufs=1, space="DRAM")
    dram = dram_cm.__enter__()

    # ---------- constants to SBUF ----------
    w1c_sb = const.tile([81, 256], F32, tag="w1c", name="w1c")
    nc.sync.dma_start(w1c_sb[:], P["w1c"][:])
    b1c_sb = [const.tile([128, 1], F32, tag=f"b1c{h}", name=f"b1c{h}") for h in range(2)]
    bp2_sb = [const.tile([128, 1], F32, tag=f"bp2{h}", name=f"bp2{h}") for h in range(2)]
    for h in range(2):
        nc.sync.dma_start(b1c_sb[h][:], P["b1c"][h * 128:(h + 1) * 128, :])
        nc.sync.dma_start(bp2_sb[h][:], P["bp2"][h * 128:(h + 1) * 128, :])
    m2_sb = const.tile([128, BL], F32, tag="m2", name="m2")
    nc.sync.dma_start(m2_sb[:], P["m2"][:])
    m4_sb = const.tile([128, 4], F32, tag="m4", name="m4")
    nc.sync.dma_start(m4_sb[:], P["m4"][:])
    ones32_sb = const.tile([BL, 1], F32, tag="ones32", name="ones32")
    nc.sync.dma_start(ones32_sb[:], P["ones32"][:])
    onesrow_sb = const.tile([1, 128], BF, tag="onesrow", name="onesrow")
    nc.sync.dma_start(onesrow_sb[:], P["onesrow"][:])
    id32_sb = const.tile([32, 32], F32, tag="id32", name="id32")
    nc.sync.dma_start(id32_sb[:], P["id32"][:])
    w1ta_sb = const.tile([128, 512], F32, tag="w1ta", name="w1ta")
    nc.sync.dma_start(w1ta_sb[:], P["w1t"][0:128, :])
    w1tb_sb = const.tile([32, 512], F32, tag="w1tb", name="w1tb")
    nc.sync.dma_start(w1tb_sb[:], P["w1t"][128:160, :])
    b1d_sb = [const.tile([128, 1], F32, tag=f"b1d{i}", name=f"b1d{i}") for i in range(4)]
    for i in range(4):
        nc.sync.dma_start(b1d_sb[i][:], P["b1d"][i * 128:(i + 1) * 128, :])
    b2d_sb = [const.tile([128, 1], F32, tag=f"b2d{i}", name=f"b2d{i}") for i in range(8)]
    for i in range(8):
        nc.sync.dma_start(b2d_sb[i][:], P["b2d"][i * 128:(i + 1) * 128, :])
    b3s_sb = const.tile([1, RECL], BF, tag="b3s", name="b3s")
    nc.sync.dma_start(b3s_sb[:], P["b3s"][:])

    # persistent mid-size tiles
    uhat_sb = const.tile([128, 20480], BF, tag="uhat", name="uhat")        # [(jm,b),(m,rr,c,o)]
    xT_sb = const.tile([128, 1024], BF, tag="xT", name="xT")             # [(row%128),(chunk,b)]
    h1T_sb = const.tile([128, 128], F32, tag="h1T", name="h1T")           # [f1%128,(fc,b)]
    h2T_sb = const.tile([128, 256], BF, tag="h2T", name="h2T")            # [f2%128,(gc,b)]
    xdram = dram.tile([2, 128, 512], BF, tag="xdram", name="xdram")
    vdram = dram.tile([BL, 160], F32, tag="vdram", name="vdram")
    ar_in = dram.tile([4, 1280], F32, tag="ar_in", name="ar_in")
    ar_out = dram.tile([4, 1280], F32, tag="ar_out", name="ar_out")
    bflat_dram = dram.tile([4, 1280], F32, tag="bflat", name="bflat")
    c2_dram = dram.tile([10, 512], F32, tag="c2d", name="c2d")
    z_in = dram.tile([1, 16], F32, tag="z_in", name="z_in")
    z_out = dram.tile([1, 16], F32, tag="z_out", name="z_out")
    h2loc = dram.tile([8, 128, BL], BF, tag="h2loc", name="h2loc")
    h2all = dram.tile([NCORES, 8, 128, BL], BF, tag="h2all", name="h2all")

    # =================== conv1 + primary caps ===================
    with tc.tile_pool(name="front", bufs=1) as front, \
         tc.tile_pool(name="wp2p", bufs=4) as wp2p, \
         tc.tile_pool(name="ps_f", bufs=2, space="PSUM") as ps_f:
        # im2col patches for conv1: [81,(yh,xh,b)]  (9 DMAs, one per dy)
        pat1 = front.tile([81, 4608], F32, tag="pat1", name="pat1")
        nc.sync.dma_start(pat1[:], P["pat1h"][:])
        H = [front.tile([128, 4608], BF, tag=f"H{h}", name=f"H{h}") for h in range(2)]
        for h in range(2):
            for w in range(9):
                ps = ps_f.tile([128, 512], F32, tag="c1ps", name="c1ps")
                nc.tensor.matmul(ps[:], w1c_sb[:, h * 128:(h + 1) * 128],
                                 pat1[:, w * 512:(w + 1) * 512],
                                 start=True, stop=True)
                nc.scalar.activation(H[h][:, w * 512:(w + 1) * 512], ps[:],
                                     ACTF.Relu, bias=b1c_sb[h][:], scale=1.0)
        # primary caps conv: K=(dy,dx,ci) 162 chunks of 128; N=(y,x,b)=512
        U = [front.tile([128, 512], F32, tag=f"U{h}", name=f"U{h}") for h in range(2)]
        psU = [ps_f.tile([128, 512], F32, tag=f"Ups{h}", name=f"Ups{h}", bufs=1) for h in range(2)]
        for t in range(162):
            dy, r = divmod(t, 18)
            dx, cih = divmod(r, 2)
            wt = wp2p.tile([128, 256], BF, tag="wp2t", name="wp2t")
            nc.sync.dma_start(wt[:], P["wp2"][t * 128:(t + 1) * 128, :])
            rhs = H[cih][:].rearrange("p (y x b) -> p y x b", y=12, x=12)[
                :, dy:dy + 4, dx:dx + 4, :]
            for h in range(2):
                nc.tensor.matmul(psU[h][:], wt[:, h * 128:(h + 1) * 128], rhs,
                                 start=(t == 0), stop=(t == 161))
        for h in range(2):
            nc.scalar.activation(U[h][:], psU[h][:], ACTF.Identity,
                                 bias=bp2_sb[h][:], scale=1.0)

        # =================== squash -> x (bf16), to DRAM, reload transposed ===
        usq = front.tile([128, 512], F32, tag="usq", name="usq")
        sn = front.tile([128, 64], F32, tag="sn", name="sn")
        g = front.tile([128, 64], F32, tag="g", name="g")
        gt = front.tile([128, 64], F32, tag="gt", name="gt")
        X = front.tile([128, 512], BF, tag="X", name="X")
        for h in range(2):
            nc.vector.tensor_tensor(usq[:], U[h][:], U[h][:], op=ALU.mult)
            uview = usq[:].rearrange("p (g i b) -> p g b i", g=2, i=8)
            nc.vector.tensor_reduce(sn[:].rearrange("p (g b) -> p g b", g=2),
                                    uview, axis=AX.X, op=ALU.add)
            nc.scalar.activation(gt[:], sn[:], ACTF.Sqrt)
            nc.vector.tensor_scalar_add(g[:], sn[:], 1.0)
            nc.vector.reciprocal(g[:], g[:])
            nc.vector.tensor_tensor(g[:], g[:], gt[:], op=ALU.mult)
            gb = g[:].rearrange("p (g b) -> p g b", g=2).unsqueeze(2).broadcast_to(
                [128, 2, 8, BL])
            nc.vector.tensor_tensor(X[:].rearrange("p (g i b) -> p g i b", g=2, i=8),
                                    U[h][:].rearrange("p (g i b) -> p g i b", g=2, i=8),
                                    gb, op=ALU.mult)
            nc.sync.dma_start(xdram[h], X[:])
        xsrc = mkap(xdram[:], 0, [[32, 128], [4096, 32], [1, 32]])
        nc.sync.dma_start(xT_sb[:], xsrc)

    # =================== u_hat: 128 windows of 4 routes ===================
    with tc.tile_pool(name="w2sp", bufs=1) as w2sp, \
         tc.tile_pool(name="ps_u", bufs=3, space="PSUM") as ps_u:
        w2s_sb = w2sp.tile([128, 20480], BF, tag="w2s", name="w2s")
        nc.sync.dma_start(w2s_sb[:], P["w2s"][:])
        for m in range(32):
            pst = [ps_u.tile([128, 320], F32, tag=f"ups{q}", name=f"ups{q}") for q in range(2)]
            for q in range(2):
                for jm in range(4):
                    nc.tensor.matmul(
                        pst[q][32 * jm:32 * (jm + 1), :],
                        xT_sb[32 * jm:32 * (jm + 1), m * 32:(m + 1) * 32],
                        w2s_sb[32 * jm:32 * (jm + 1),
                               m * 640 + q * 320:m * 640 + (q + 1) * 320],
                        start=True, stop=True, tile_position=(32 * jm, 32 * jm))
                nc.scalar.activation(
                    uhat_sb[:, m * 640 + q * 320:m * 640 + (q + 1) * 320],
                    pst[q][:], ACTF.Copy)

    # =================== routing (3 iters, 2 AllReduce) ===================
    with tc.tile_pool(name="route", bufs=1) as rt, \
         tc.tile_pool(name="ps_r", bufs=1, space="PSUM") as ps_r:
        tmp = rt.tile([128, 20480], BF, tag="tmp", name="tmp")
        s_part = rt.tile([128, 160], F32, tag="s_part", name="s_part")
        s_sb = rt.tile([BL, 160], F32, tag="s_sb", name="s_sb")
        sq = rt.tile([BL, 160], F32, tag="sq", name="sq")
        num = rt.tile([BL, 160], F32, tag="num", name="num")
        dn = rt.tile([BL, 160], F32, tag="dn", name="dn")
        v_sb = rt.tile([BL, 160], F32, tag="v_sb", name="v_sb")
        vrep = rt.tile([128, 160], BF, tag="vrep", name="vrep")
        crep = rt.tile([128, 1280], BF, tag="crep", name="crep")
        a_sb = rt.tile([128, 1280], F32, tag="a_sb", name="a_sb")
        braw = rt.tile([4, 1280], F32, tag="braw", name="braw")
        arres = rt.tile([4, 1280], F32, tag="arres", name="arres")
        b_acc = rt.tile([4, 1280], F32, tag="b_acc", name="b_acc")
        csf = rt.tile([10, 512], F32, tag="csf", name="csf")
        rmax = rt.tile([10, 1], F32, tag="rmax", name="rmax")
        nbias = rt.tile([10, 1], F32, tag="nbias", name="nbias")
        esum = rt.tile([10, 1], F32, tag="esum", name="esum")
        c_sb = rt.tile([10, 512], F32, tag="c_sb", name="c_sb")

        uh5 = uhat_sb[:].rearrange("p (m rr c o) -> p m rr c o", m=32, rr=4, c=10)
        tmp5 = tmp[:].rearrange("p (m rr c o) -> p m rr c o", m=32, rr=4, c=10)

        for it in range(3):
            # ---- s_part [(jm,b),(c,o)] ----
            if it == 0:
                red_in = uhat_sb[:].rearrange("p (m rr c o) -> p c o m rr",
                                              m=32, rr=4, c=10)
                nc.vector.tensor_reduce(
                    s_part[:].rearrange("p (c o) -> p c o", c=10),
                    red_in, axis=AX.XY, op=ALU.add)
            else:
                cb = crep[:].rearrange("p (c m rr) -> p m rr c", c=10, m=32
                                       ).unsqueeze(4).broadcast_to([128, 32, 4, 10, 16])
                nc.vector.tensor_tensor(tmp5, uh5, cb, op=ALU.mult)
                red_in = tmp[:].rearrange("p (m rr c o) -> p c o m rr",
                                          m=32, rr=4, c=10)
                nc.vector.tensor_reduce(
                    s_part[:].rearrange("p (c o) -> p c o", c=10),
                    red_in, axis=AX.XY, op=ALU.add)
            psS = ps_r.tile([BL, 160], F32, tag="psS", name="psS")
            nc.tensor.matmul(psS[:], m2_sb[:], s_part[:], start=True, stop=True)
            nc.scalar.mul(s_sb[:], psS[:], (1.0 / 512.0) if it == 0 else 1.0)
            # ---- elementwise squash: v = sq*s/((1+sq)*sqrt(sq)) ----
            nc.vector.tensor_tensor(sq[:], s_sb[:], s_sb[:], op=ALU.mult)
            nc.vector.tensor_tensor(num[:], sq[:], s_sb[:], op=ALU.mult)
            nc.vector.tensor_scalar_add(dn[:], sq[:], 1.0)
            nc.scalar.activation(sq[:], sq[:], ACTF.Sqrt)  # sq <- sqrt(sq)=|s|
            nc.vector.tensor_tensor(dn[:], dn[:], sq[:], op=ALU.mult)
            nc.vector.reciprocal(dn[:], dn[:])
            nc.vector.tensor_tensor(v_sb[:], num[:], dn[:], op=ALU.mult)

            if it == 2:
                break
            # ---- a = <u_hat, v>_o ; b_delta = mean_b a (via AllReduce) ----
            nc.sync.dma_start(vdram[:], v_sb[:])
            for jm in range(4):
                vsrc = mkap(vdram[:], 0, [[160, 32], [1, 160]])
                nc.gpsimd.dma_start(vrep[32 * jm:32 * (jm + 1), :], vsrc)
            vb = vrep[:].rearrange("p (c o) -> p c o", c=10).unsqueeze(1)\
                .unsqueeze(1).broadcast_to([128, 32, 4, 10, 16])
            nc.vector.tensor_tensor(tmp5, uh5, vb, op=ALU.mult)
            nc.vector.tensor_reduce(
                a_sb[:].rearrange("p (c m rr) -> p m rr c", c=10, m=32),
                tmp5, axis=AX.X, op=ALU.add)
            for seg, (off, ln) in enumerate([(0, 512), (512, 512), (1024, 256)]):
                psb = ps_r.tile([4, 512], F32, tag="psb", name="psb")
                nc.tensor.matmul(psb[:, :ln], m4_sb[:],
                                 a_sb[:, off:off + ln],
                                 start=True, stop=True)
                nc.scalar.activation(braw[:, off:off + ln], psb[:, :ln], ACTF.Copy)
            nc.sync.dma_start(ar_in[:], braw[:])
            nc.gpsimd.collective_compute(
                "AllReduce", ALU.add, replica_groups=rg,
                ins=[ar_in[:].opt()], outs=[ar_out[:].opt()])
            nc.sync.dma_start(arres[:], ar_out[:])
            if it == 0:
                nc.vector.tensor_copy(b_acc[:], arres[:])
            else:
                nc.vector.tensor_tensor(b_acc[:], b_acc[:], arres[:], op=ALU.add)
            # ---- write b state transposed [10,512], softmax over routes ----
            nc.sync.dma_start(bflat_dram[:], b_acc[:])
            csrc2 = mkap(bflat_dram[:], 0, [[128, 10], [1280, 4], [1, 128]])
            nc.sync.dma_start(csf[:].rearrange("p (jm mr) -> p jm mr", jm=4), csrc2)
            nc.vector.tensor_reduce(rmax[:], csf[:], axis=AX.X, op=ALU.max)
            nc.scalar.mul(nbias[:], rmax[:], -1.0 / 256.0)
            nc.scalar.activation(c_sb[:], csf[:], ACTF.Exp,
                                 bias=nbias[:], scale=1.0 / 256.0)
            nc.vector.tensor_reduce(esum[:], c_sb[:], axis=AX.X, op=ALU.add)
            nc.vector.reciprocal(esum[:], esum[:])
            nc.vector.tensor_scalar_mul(c_sb[:], c_sb[:], esum[:])
            nc.sync.dma_start(c2_dram[:], c_sb[:])
            for jm in range(4):
                csrc = mkap(c2_dram[:], jm * 128, [[0, 32], [512, 10], [1, 128]])
                nc.gpsimd.dma_start(crep[32 * jm:32 * (jm + 1), :], csrc)

        # =================== classes/argmax/mask -> flat ===================
        nc.vector.tensor_tensor(sq[:], v_sb[:], v_sb[:], op=ALU.mult)
        cl = rt.tile([BL, 10], F32, tag="cl", name="cl")
        nc.vector.tensor_reduce(cl[:], sq[:].rearrange("p (c o) -> p c o", c=10),
                                axis=AX.X, op=ALU.add)
        nc.scalar.activation(cl[:], cl[:], ACTF.Sqrt)
        ecl = rt.tile([BL, 10], F32, tag="ecl", name="ecl")
        nc.scalar.activation(ecl[:], cl[:], ACTF.Exp)
        psZ = ps_r.tile([1, 16], F32, tag="psZ", name="psZ")
        nc.tensor.matmul(psZ[:, :10], ones32_sb[:], ecl[:], start=True, stop=True)
        zrow = rt.tile([1, 16], F32, tag="zrow", name="zrow")
        nc.vector.memset(zrow[:], 0)
        nc.scalar.activation(zrow[:, :10], psZ[:, :10], ACTF.Copy)
        nc.sync.dma_start(z_in[:], zrow[:])
        nc.gpsimd.collective_compute(
            "AllReduce", ALU.add, replica_groups=rg,
            ins=[z_in[:].opt()], outs=[z_out[:].opt()])
        zfull = rt.tile([BL, 10], F32, tag="zfull", name="zfull")
        nc.gpsimd.dma_start(zfull[:], mkap(z_out[:], 0, [[0, BL], [1, 10]]))
        nc.vector.reciprocal(zfull[:], zfull[:])
        tpr = rt.tile([BL, 10], F32, tag="tpr", name="tpr")
        nc.vector.tensor_tensor(tpr[:], ecl[:], zfull[:], op=ALU.mult)
        tmax = rt.tile([BL, 1], F32, tag="tmax", name="tmax")
        nc.vector.tensor_reduce(tmax[:], tpr[:], axis=AX.X, op=ALU.max)
        mask = rt.tile([BL, 10], F32, tag="mask", name="mask")
        nc.vector.tensor_scalar(mask[:], tpr[:], tmax[:], None, op0=ALU.is_equal)
        flat = rt.tile([BL, 160], F32, tag="flat", name="flat")
        mb = mask[:].unsqueeze(2).broadcast_to([BL, 10, 16])
        nc.vector.tensor_tensor(flat[:].rearrange("p (c o) -> p c o", c=10),
                                v_sb[:].rearrange("p (c o) -> p c o", c=10),
                                mb, op=ALU.mult)

        # =================== decoder fc1 fc2 (transposed) ===================
        psT = ps_r.tile([128, 32], F32, tag="psT", name="psT")
        nc.tensor.transpose(psT[:], flat[:, 0:128], id32_sb[:])
        fTa = rt.tile([128, 32], F32, tag="fTa", name="fTa")
        nc.scalar.activation(fTa[:], psT[:], ACTF.Copy)
        psT2 = ps_r.tile([32, 32], F32, tag="psT2", name="psT2")
        nc.tensor.transpose(psT2[:], flat[:, 128:160], id32_sb[:])
        fTb = rt.tile([32, 32], F32, tag="fTb", name="fTb")
        nc.scalar.activation(fTb[:], psT2[:], ACTF.Copy)
        for fc in range(4):
            ps1 = ps_r.tile([128, 32], F32, tag="ps1", name="ps1")
            nc.tensor.matmul(ps1[:], w1ta_sb[:, fc * 128:(fc + 1) * 128], fTa[:],
                             start=True, stop=False)
            nc.tensor.matmul(ps1[:], w1tb_sb[:, fc * 128:(fc + 1) * 128], fTb[:],
                             start=False, stop=True)
            nc.scalar.activation(h1T_sb[:, fc * 32:(fc + 1) * 32], ps1[:],
                                 ACTF.Relu, bias=b1d_sb[fc][:], scale=1.0)

    with tc.tile_pool(name="decw", bufs=1) as decw, \
         tc.tile_pool(name="ps_d", bufs=2, space="PSUM") as ps_d:
        w2t_sb = [decw.tile([128, 1024], F32, tag=f"w2t{i}", name=f"w2t{i}") for i in range(4)]
        for i in range(4):
            nc.sync.dma_start(w2t_sb[i][:], P["w2t"][i * 128:(i + 1) * 128, :])
        for gc in range(8):
            ps2 = ps_d.tile([128, 32], F32, tag="ps2", name="ps2")
            for kc in range(4):
                nc.tensor.matmul(ps2[:], w2t_sb[kc][:, gc * 128:(gc + 1) * 128],
                                 h1T_sb[:, kc * 32:(kc + 1) * 32],
                                 start=(kc == 0), stop=(kc == 3))
            nc.scalar.activation(h2T_sb[:, gc * 32:(gc + 1) * 32], ps2[:],
                                 ACTF.Relu, bias=b2d_sb[gc][:], scale=1.0)
        h2dst = mkap(h2loc[:], 0, [[BL, 128], [128 * BL, 8], [1, BL]])
        nc.sync.dma_start(h2dst, h2T_sb[:])
        nc.gpsimd.collective_compute(
            "AllGather", ALU.bypass, replica_groups=rg,
            ins=[h2loc[:].opt()], outs=[h2all[:].opt()])

    # =================== final big layer (tensor-parallel) ===================
    with tc.tile_pool(name="fin", bufs=1) as fin, \
         tc.tile_pool(name="w3p", bufs=10) as w3p, \
         tc.tile_pool(name="ps_o", bufs=4, space="PSUM") as ps_o, \
         tc.tile_pool(name="osb", bufs=4) as osbp:
        ld = [fin.tile([128, 256], BF, tag=f"ld{kc}", name=f"ld{kc}") for kc in range(8)]
        for kc in range(8):
            src = mkap(h2all[:], kc * 128 * BL,
                       [[BL, 128], [8 * 128 * BL, NCORES], [1, BL]])
            nc.sync.dma_start(ld[kc][:], src)
        for w in range(NW):
            w3tiles = []
            for kc in range(8):
                wt3 = w3p.tile([128, 512], BF, tag="w3t", name="w3t")
                nc.sync.dma_start(wt3[:], P["w3t"][kc * 128:(kc + 1) * 128,
                                                   w * 512:(w + 1) * 512])
                w3tiles.append(wt3)
            for bh in range(2):
                pso = ps_o.tile([128, 512], F32, tag="pso", name="pso")
                for kc in range(8):
                    nc.tensor.matmul(pso[:], ld[kc][:, bh * 128:(bh + 1) * 128],
                                     w3tiles[kc][:], start=(kc == 0), stop=False)
                nc.tensor.matmul(pso[:], onesrow_sb[:],
                                 b3s_sb[:, w * 512:(w + 1) * 512],
                                 start=False, stop=True)
                ot = osbp.tile([128, 512], F32, tag="ot", name="ot")
                nc.scalar.activation(ot[:], pso[:], ACTF.Sigmoid)
                nc.sync.dma_start(out_ext[bh * 128:(bh + 1) * 128,
                                          w * 512:(w + 1) * 512], ot[:])


_NC_CACHE = {}


def _host_prep(inputs):
    data = np.asarray(inputs["data"], np.float32)
    conv1_w = np.asarray(inputs["conv1_w"], np.float32)
    conv1_b = np.asarray(inputs["conv1_b"], np.float32)
    prim_w = np.asarray(inputs["prim_w"], np.float32)
    prim_b = np.asarray(inputs["prim_b"], np.float32)
    W_digit = np.asarray(inputs["W_digit"], np.float32)
    dec_w1 = np.asarray(inputs["dec_w1"], np.float32)
    dec_b1 = np.asarray(inputs["dec_b1"], np.float32)
    dec_w2 = np.asarray(inputs["dec_w2"], np.float32)
    dec_b2 = np.asarray(inputs["dec_b2"], np.float32)
    dec_w3 = np.asarray(inputs["dec_w3"], np.float32)
    dec_b3 = np.asarray(inputs["dec_b3"], np.float32)

    w1c = np.ascontiguousarray(conv1_w[:, 0].transpose(1, 2, 0).reshape(81, 256))
    wp2 = np.ascontiguousarray(
        prim_w.transpose(2, 3, 1, 0).reshape(20736, 256)).astype(BF16)
    # W2[r,i,co] ; w2stack [128,(m,rr2,co)] block-diagonal over rr
    W2 = np.ascontiguousarray(W_digit.transpose(0, 3, 1, 2).reshape(512, 8, 160))
    w2s = np.zeros((128, 32, 4, 160), np.float32)
    marr = np.arange(32)
    for jm in range(4):
        for rr in range(4):
            for i in range(8):
                w2s[32 * jm + rr * 8 + i, :, rr, :] = W2[4 * (4 * marr + jm) + rr, i, :]
    w2s = w2s.reshape(128, 20480).astype(BF16)
    m2 = np.tile(np.eye(32, dtype=np.float32), (4, 1))
    m4 = np.repeat(np.eye(4, dtype=np.float32), 32, axis=0)
    w1t = np.ascontiguousarray(dec_w1.T)
    w2t = np.ascontiguousarray(dec_w2.T)
    w3t = np.ascontiguousarray(dec_w3.T).astype(BF16)

    common = dict(
        w1c=w1c, b1c=conv1_b.reshape(256, 1), wp2=wp2,
        bp2=prim_b.reshape(256, 1), w2s=w2s, m2=m2, m4=m4,
        ones32=np.ones((32, 1), np.float32),
        onesrow=np.ones((1, 128), np.float32).astype(BF16),
        id32=np.eye(32, dtype=np.float32),
        w1t=w1t, b1d=dec_b1.reshape(512, 1),
        w2t=w2t, b2d=dec_b2.reshape(1024, 1),
    )
    in_maps = []
    for c in range(NCORES):
        m = dict(common)
        sw = np.lib.stride_tricks.sliding_window_view(
            data[c * BL:(c + 1) * BL, 0], (9, 9), axis=(1, 2))
        m["pat1h"] = np.ascontiguousarray(
            sw.transpose(3, 4, 1, 2, 0).reshape(81, 4608))
        m["w3t"] = np.ascontiguousarray(w3t[:, c * RECL:(c + 1) * RECL])
        m["b3s"] = dec_b3[c * RECL:(c + 1) * RECL].reshape(1, RECL).astype(BF16)
        in_maps.append(m)
    return in_maps


def kernel(**inputs):
    if "nc" not in _NC_CACHE:
        _NC_CACHE["nc"] = build_program()
    nc = _NC_CACHE["nc"]
    in_maps = _host_prep(inputs)
    res = run_bass_kernel_spmd(nc, in_maps, list(range(NCORES)))
    outs = [res.results[c]["out"] for c in range(NCORES)]
    rec = np.concatenate(outs, axis=1).astype(np.float32)
    return rec.reshape(B, 256, 20, 20)



# revision 4
# speedup vs baseline: 1.9912x; 1.9912x over previous
import sys

sys.path.insert(0, "/opt/trn_rl_repo")

import numpy as np
import ml_dtypes

import concourse.bass as bass
import concourse.mybir as mybir
import concourse.tile as tile
from concourse import bacc
from concourse.bass_utils import run_bass_kernel_spmd

BF16 = ml_dtypes.bfloat16
FP8 = ml_dtypes.float8_e4m3
F32 = mybir.dt.float32
BF = mybir.dt.bfloat16
F8 = mybir.dt.float8e4
ALU = mybir.AluOpType
ACTF = mybir.ActivationFunctionType
AX = mybir.AxisListType
PM = mybir.MatmulPerfMode

NCORES = 8
B = 256
BL = B // NCORES          # 32 local batch
REC = 102400
RECL = REC // NCORES      # 12800 local output cols
NW = RECL // 512          # 25 output windows

S_H = 32.0                # conv1-activation fp8 scale
S_WP = 4096.0             # primary-caps weight fp8 scale
S_H2 = 2.0 ** 23          # h2 fp8 scale
S_W3 = 1024.0             # dec_w3 fp8 scale


def mkap(t, offset, dims):
    """Manual access pattern: dims = [[stride, count], ...] (partition dim first)."""
    return bass.AP(tensor=t.tensor if isinstance(t, bass.AP) else t, offset=offset, ap=dims)


def build_program():
    nc = bacc.Bacc(None, num_devices=NCORES)
    rg = [list(range(NCORES))]

    P = {}
    P["pat1h"] = nc.declare_dram_parameter("pat1h", [81, 4608], BF, isOutput=False)
    P["w1c"] = nc.declare_dram_parameter("w1c", [81, 256], BF, isOutput=False)
    P["b1s"] = nc.declare_dram_parameter("b1s", [256, 1], F32, isOutput=False)
    P["wp2q"] = nc.declare_dram_parameter("wp2q", [20736, 256], F8, isOutput=False)
    P["bp2"] = nc.declare_dram_parameter("bp2", [256, 1], F32, isOutput=False)
    P["wfull"] = nc.declare_dram_parameter("wfull", [128, 5120], BF, isOutput=False)
    P["ones8"] = nc.declare_dram_parameter("ones8", [128, 16], F32, isOutput=False)
    P["id128"] = nc.declare_dram_parameter("id128", [128, 128], BF, isOutput=False)
    P["w1t"] = nc.declare_dram_parameter("w1t", [160, 512], BF, isOutput=False)
    P["b1d"] = nc.declare_dram_parameter("b1d", [512, 1], F32, isOutput=False)
    P["w2t"] = nc.declare_dram_parameter("w2t", [512, 1024], BF, isOutput=False)
    P["b2s"] = nc.declare_dram_parameter("b2s", [1024, 1], F32, isOutput=False)
    P["w3q"] = nc.declare_dram_parameter("w3q", [128, 102400], F8, isOutput=False)
    P["b3q"] = nc.declare_dram_parameter("b3q", [1, RECL], BF, isOutput=False)
    P["onesrow"] = nc.declare_dram_parameter("onesrow", [1, 128], BF, isOutput=False)
    P["ones128"] = nc.declare_dram_parameter("ones128", [128, 1], F32, isOutput=False)
    out_ext = nc.declare_dram_parameter("out", [B, RECL], BF, isOutput=True)

    with tile.TileContext(nc) as tc:
        _body(nc, tc, P, out_ext, rg)
    nc.compile()
    return nc


def _body(nc, tc, P, out_ext, rg):
    es = tc.tile_pool(name="const", bufs=1)
    const = es.__enter__()
    dram_cm = tc.tile_pool(name="dram", bufs=1, space="DRAM")
    dram = dram_cm.__enter__()

    # ---------- DRAM scratch ----------
    warm_in = dram.tile([1, 16], F32, tag="warm_in", name="warm_in")
    warm_out = dram.tile([1, 16], F32, tag="warm_out", name="warm_out")
    xdram = dram.tile([2, 128, 512], BF, tag="xdram", name="xdram")
    ar_in = dram.tile([16, 320], F32, tag="ar_in", name="ar_in")
    ar_out = dram.tile([16, 320], F32, tag="ar_out", name="ar_out")
    bflat = dram.tile([16, 320], F32, tag="bflat", name="bflat")
    c2d = dram.tile([10, 512], BF, tag="c2d", name="c2d")
    z_in = dram.tile([1, 16], F32, tag="z_in", name="z_in")
    z_out = dram.tile([1, 16], F32, tag="z_out", name="z_out")
    vin = dram.tile([BL, 160], F32, tag="vin", name="vin")
    vall = dram.tile([NCORES, BL, 160], F32, tag="vall", name="vall")

    # ---------- warmup collective (absorb first-collective setup cost) ----------
    zw = const.tile([1, 16], F32, tag="zw", name="zw")
    nc.vector.memset(zw[:], 0)
    nc.sync.dma_start(warm_in[:], zw[:])
    nc.gpsimd.collective_compute(
        "AllReduce", ALU.add, replica_groups=rg,
        ins=[warm_in[:].opt()], outs=[warm_out[:].opt()])

    # ---------- constants to SBUF ----------
    w1c_sb = const.tile([81, 256], BF, tag="w1c", name="w1c")
    nc.sync.dma_start(w1c_sb[:], P["w1c"][:])
    b1s_sb = [const.tile([128, 1], F32, tag=f"b1s{h}", name=f"b1s{h}") for h in range(2)]
    bp2_sb = [const.tile([128, 1], F32, tag=f"bp2{h}", name=f"bp2{h}") for h in range(2)]
    for h in range(2):
        nc.sync.dma_start(b1s_sb[h][:], P["b1s"][h * 128:(h + 1) * 128, :])
        nc.sync.dma_start(bp2_sb[h][:], P["bp2"][h * 128:(h + 1) * 128, :])
    wfull_sb = const.tile([128, 5120], BF, tag="wfull", name="wfull")
    nc.sync.dma_start(wfull_sb[:], P["wfull"][:])
    ones8_sb = const.tile([128, 16], F32, tag="ones8", name="ones8")
    nc.sync.dma_start(ones8_sb[:], P["ones8"][:])
    id128_sb = const.tile([128, 128], BF, tag="id128", name="id128")
    nc.sync.dma_start(id128_sb[:], P["id128"][:])
    w1ta_sb = const.tile([128, 512], BF, tag="w1ta", name="w1ta")
    nc.sync.dma_start(w1ta_sb[:], P["w1t"][0:128, :])
    w1tb_sb = const.tile([32, 512], BF, tag="w1tb", name="w1tb")
    nc.sync.dma_start(w1tb_sb[:], P["w1t"][128:160, :])
    b1d_sb = [const.tile([128, 1], F32, tag=f"b1d{i}", name=f"b1d{i}") for i in range(4)]
    for i in range(4):
        nc.sync.dma_start(b1d_sb[i][:], P["b1d"][i * 128:(i + 1) * 128, :])
    w2t_sb = [const.tile([128, 1024], BF, tag=f"w2t{i}", name=f"w2t{i}") for i in range(4)]
    for i in range(4):
        nc.sync.dma_start(w2t_sb[i][:], P["w2t"][i * 128:(i + 1) * 128, :])
    b2s_sb = [const.tile([128, 1], F32, tag=f"b2s{i}", name=f"b2s{i}") for i in range(8)]
    for i in range(8):
        nc.sync.dma_start(b2s_sb[i][:], P["b2s"][i * 128:(i + 1) * 128, :])
    onesrow_sb = const.tile([1, 128], BF, tag="onesrow", name="onesrow")
    nc.sync.dma_start(onesrow_sb[:], P["onesrow"][:])
    ones128_sb = const.tile([128, 1], F32, tag="ones128", name="ones128")
    nc.sync.dma_start(ones128_sb[:], P["ones128"][:])
    b3q_sb = const.tile([1, RECL], BF, tag="b3q", name="b3q")
    nc.sync.dma_start(b3q_sb[:], P["b3q"][:])
    # big fp8 decoder weight: prefetch fully into SBUF (overlaps front+routing)
    w3q_sb = const.tile([128, 102400], F8, tag="w3q", name="w3q")
    nc.sync.dma_start(w3q_sb[:], P["w3q"][:])

    # persistent mid-size tiles
    xT_sb = const.tile([128, 1024], BF, tag="xT", name="xT")      # [(cl,g,i), (chunk,b)]
    x2_sb = const.tile([BL, 4096], BF, tag="x2", name="x2")       # [b, (chunk,cl,g,i)]
    G_sb = const.tile([128, 5120], BF, tag="G", name="G")         # [(cl,g,i), (chunk,cd,o)]
    wc_sb = const.tile([128, 5120], BF, tag="wc", name="wc")      # c-weighted W / prod scratch
    h2q_sb = const.tile([128, 8, 256], F8, tag="h2q", name="h2q")  # [feat%128, kc, b]

    # =================== conv1 + primary caps (fp8 DoubleRow) ===================
    with tc.tile_pool(name="front", bufs=1) as front, \
         tc.tile_pool(name="wp2p", bufs=4) as wp2p, \
         tc.tile_pool(name="ps_f", bufs=2, space="PSUM") as ps_f:
        pat1 = front.tile([81, 4608], BF, tag="pat1", name="pat1")
        nc.sync.dma_start(pat1[:], P["pat1h"][:])
        H = front.tile([128, 2, 4608], F8, tag="H", name="H")
        for h in range(2):
            for w in range(9):
                ps = ps_f.tile([128, 512], F32, tag="c1ps", name="c1ps")
                nc.tensor.matmul(ps[:], w1c_sb[:, h * 128:(h + 1) * 128],
                                 pat1[:, w * 512:(w + 1) * 512],
                                 start=True, stop=True)
                nc.scalar.activation(H[:, h, w * 512:(w + 1) * 512], ps[:],
                                     ACTF.Relu, bias=b1s_sb[h][:], scale=S_H)
        U = [front.tile([128, 512], F32, tag=f"U{h}", name=f"U{h}") for h in range(2)]
        psU = [ps_f.tile([128, 512], F32, tag=f"Ups{h}", name=f"Ups{h}", bufs=1) for h in range(2)]
        Hv = H[:].rearrange("p c (y x b) -> p c y x b", y=12, x=12)
        for u in range(81):
            dy, dx = divmod(u, 9)
            wt = wp2p.tile([128, 2, 256], F8, tag="wp2t", name="wp2t")
            nc.sync.dma_start(wt[:], mkap(P["wp2q"], u * 65536,
                                          [[256, 128], [32768, 2], [1, 256]]))
            rhs = Hv[:, :, dy:dy + 4, dx:dx + 4, :]
            for h in range(2):
                nc.tensor.matmul(psU[h][:], wt[:, :, h * 128:(h + 1) * 128], rhs,
                                 start=(u == 0), stop=(u == 80),
                                 perf_mode=PM.DoubleRow)
        for h in range(2):
            nc.scalar.activation(U[h][:], psU[h][:], ACTF.Identity,
                                 bias=bp2_sb[h][:], scale=1.0 / (S_H * S_WP))

        # ---- squash -> x (bf16), to DRAM, reload transposed ----
        usq = front.tile([128, 512], F32, tag="usq", name="usq")
        sn = front.tile([128, 64], F32, tag="sn", name="sn")
        g = front.tile([128, 64], F32, tag="g", name="g")
        gt = front.tile([128, 64], F32, tag="gt", name="gt")
        X = front.tile([128, 512], BF, tag="X", name="X")
        for h in range(2):
            nc.vector.tensor_tensor(usq[:], U[h][:], U[h][:], op=ALU.mult)
            uview = usq[:].rearrange("p (g i b) -> p g b i", g=2, i=8)
            nc.vector.tensor_reduce(sn[:].rearrange("p (g b) -> p g b", g=2),
                                    uview, axis=AX.X, op=ALU.add)
            nc.scalar.activation(gt[:], sn[:], ACTF.Sqrt)
            nc.vector.tensor_scalar_add(g[:], sn[:], 1.0)
            nc.vector.reciprocal(g[:], g[:])
            nc.vector.tensor_tensor(g[:], g[:], gt[:], op=ALU.mult)
            gb = g[:].rearrange("p (g b) -> p g b", g=2).unsqueeze(2).broadcast_to(
                [128, 2, 8, BL])
            nc.vector.tensor_tensor(X[:].rearrange("p (g i b) -> p g i b", g=2, i=8),
                                    U[h][:].rearrange("p (g i b) -> p g i b", g=2, i=8),
                                    gb, op=ALU.mult)
            nc.sync.dma_start(xdram[h], X[:])
        xsrc = mkap(xdram[:], 0, [[32, 128], [4096, 32], [1, 32]])
        nc.sync.dma_start(xT_sb[:], xsrc)

    # =================== routing (3 iters, matmul-factored) ===================
    with tc.tile_pool(name="route", bufs=1) as rt, \
         tc.tile_pool(name="ps_r", bufs=1, space="PSUM") as ps_r:
        # x2 via 32 tensor transposes of xT chunks
        for j in range(32):
            psT = ps_r.tile([32, 128], BF, tag="psT", name="psT", bufs=2)
            nc.tensor.transpose(psT[:], xT_sb[:, j * 32:(j + 1) * 32], id128_sb[:])
            nc.scalar.activation(x2_sb[:, j * 128:(j + 1) * 128], psT[:], ACTF.Copy)

        s_sb = rt.tile([BL, 160], F32, tag="s_sb", name="s_sb")
        sq = rt.tile([BL, 160], F32, tag="sq", name="sq")
        num = rt.tile([BL, 160], F32, tag="num", name="num")
        dn = rt.tile([BL, 160], F32, tag="dn", name="dn")
        v_sb = rt.tile([BL, 160], F32, tag="v_sb", name="v_sb")
        vq = rt.tile([BL, 160], BF, tag="vq", name="vq")
        t1 = rt.tile([128, 320], F32, tag="t1", name="t1")
        braw = rt.tile([16, 320], F32, tag="braw", name="braw")
        b_acc = rt.tile([16, 320], F32, tag="b_acc", name="b_acc")
        arres = rt.tile([16, 320], F32, tag="arres", name="arres")
        csf = rt.tile([10, 512], F32, tag="csf", name="csf")
        rmax = rt.tile([10, 1], F32, tag="rmax", name="rmax")
        nbias = rt.tile([10, 1], F32, tag="nbias", name="nbias")
        esum = rt.tile([10, 1], F32, tag="esum", name="esum")
        c_sb = rt.tile([10, 512], F32, tag="c_sb", name="c_sb")
        c_bf = rt.tile([10, 512], BF, tag="c_bf", name="c_bf")
        crep2 = rt.tile([128, 320], BF, tag="crep2", name="crep2")

        for it in range(3):
            # ---- s = sum_r c_r u_hat  via matmul over (r,i) ----
            psS = ps_r.tile([BL, 160], F32, tag="psS", name="psS")
            if it == 0:
                for j in range(32):
                    nc.tensor.matmul(psS[:], xT_sb[:, j * 32:(j + 1) * 32],
                                     wfull_sb[:, j * 160:(j + 1) * 160],
                                     start=(j == 0), stop=(j == 31))
                nc.scalar.mul(s_sb[:], psS[:], 1.0 / 512.0)
            else:
                cv = crep2[:].rearrange("p (c m) -> p m c", c=10).unsqueeze(3)\
                    .broadcast_to([128, 32, 10, 16])
                nc.vector.tensor_tensor(
                    wc_sb[:].rearrange("p (m c o) -> p m c o", m=32, c=10),
                    wfull_sb[:].rearrange("p (m c o) -> p m c o", m=32, c=10),
                    cv, op=ALU.mult)
                for j in range(32):
                    nc.tensor.matmul(psS[:], xT_sb[:, j * 32:(j + 1) * 32],
                                     wc_sb[:, j * 160:(j + 1) * 160],
                                     start=(j == 0), stop=(j == 31))
                nc.scalar.activation(s_sb[:], psS[:], ACTF.Copy)
            # ---- elementwise squash: v = sq*s/((1+sq)*sqrt(sq)) ----
            nc.vector.tensor_tensor(sq[:], s_sb[:], s_sb[:], op=ALU.mult)
            nc.vector.tensor_tensor(num[:], sq[:], s_sb[:], op=ALU.mult)
            nc.vector.tensor_scalar_add(dn[:], sq[:], 1.0)
            nc.scalar.activation(sq[:], sq[:], ACTF.Sqrt)
            nc.vector.tensor_tensor(dn[:], dn[:], sq[:], op=ALU.mult)
            nc.vector.reciprocal(dn[:], dn[:])
            nc.vector.tensor_tensor(v_sb[:], num[:], dn[:], op=ALU.mult)

            if it == 2:
                break
            # ---- G[(r,i),(cd,o)] = sum_b x v  (32 matmuls) ----
            nc.vector.tensor_copy(vq[:], v_sb[:])
            for j in range(32):
                psG = ps_r.tile([128, 160], F32, tag="psG", name="psG", bufs=2)
                nc.tensor.matmul(psG[:], x2_sb[:, j * 128:(j + 1) * 128], vq[:],
                                 start=True, stop=True)
                nc.scalar.activation(G_sb[:, j * 160:(j + 1) * 160], psG[:], ACTF.Copy)
            # ---- a_mean = sum_{o,i} W . G ----
            nc.vector.tensor_tensor(wc_sb[:], G_sb[:], wfull_sb[:], op=ALU.mult)
            nc.vector.tensor_reduce(
                t1[:].rearrange("p (m c) -> p m c", m=32),
                wc_sb[:].rearrange("p (m c o) -> p m c o", m=32, c=10),
                axis=AX.X, op=ALU.add)
            psA = ps_r.tile([16, 320], F32, tag="psA", name="psA")
            nc.tensor.matmul(psA[:], ones8_sb[:],
                             t1[:].rearrange("p (m c) -> p c m", m=32),
                             start=True, stop=True)
            nc.scalar.activation(braw[:], psA[:], ACTF.Copy)
            nc.sync.dma_start(ar_in[:], braw[:])
            nc.gpsimd.collective_compute(
                "AllReduce", ALU.add, replica_groups=rg,
                ins=[ar_in[:].opt()], outs=[ar_out[:].opt()])
            nc.sync.dma_start(arres[:], ar_out[:])
            if it == 0:
                nc.vector.tensor_copy(b_acc[:], arres[:])
            else:
                nc.vector.tensor_tensor(b_acc[:], b_acc[:], arres[:], op=ALU.add)
            # ---- softmax over routes: b [16,(cd,m)] -> csf [10,(q,m)] ----
            nc.sync.dma_start(bflat[:], b_acc[:])
            nc.sync.dma_start(csf[:], mkap(bflat[:], 0, [[32, 10], [320, 16], [1, 32]]))
            nc.vector.tensor_reduce(rmax[:], csf[:], axis=AX.X, op=ALU.max)
            nc.scalar.mul(nbias[:], rmax[:], -1.0)
            nc.scalar.activation(c_sb[:], csf[:], ACTF.Exp, bias=nbias[:], scale=1.0)
            nc.vector.tensor_reduce(esum[:], c_sb[:], axis=AX.X, op=ALU.add)
            nc.vector.reciprocal(esum[:], esum[:])
            nc.vector.tensor_scalar_mul(c_sb[:], c_sb[:], esum[:])
            nc.vector.tensor_copy(c_bf[:], c_sb[:])
            nc.sync.dma_start(c2d[:], c_bf[:])
            for q in range(16):
                src = mkap(c2d[:], q * 32, [[0, 8], [512, 10], [1, 32]])
                eng = nc.sync if q % 2 == 0 else nc.gpsimd
                eng.dma_start(crep2[8 * q:8 * (q + 1), :], src)

        # =================== tail: AllGather v, full-batch decoder ===========
        nc.sync.dma_start(vin[:], v_sb[:])
        nc.gpsimd.collective_compute(
            "AllGather", ALU.bypass, replica_groups=rg,
            ins=[vin[:].opt()], outs=[vall[:].opt()])

        vfull = [rt.tile([128, 160], F32, tag=f"vf{bh}", name=f"vf{bh}") for bh in range(2)]
        ecl = [rt.tile([128, 10], F32, tag=f"ecl{bh}", name=f"ecl{bh}") for bh in range(2)]
        sqf = rt.tile([128, 160], F32, tag="sqf", name="sqf")
        cl = rt.tile([128, 10], F32, tag="cl", name="cl")
        psZ = ps_r.tile([1, 16], F32, tag="psZ", name="psZ", bufs=1)
        for bh in range(2):
            nc.sync.dma_start(vfull[bh][:],
                              mkap(vall[:], bh * 128 * 160, [[160, 128], [1, 160]]))
            nc.vector.tensor_tensor(sqf[:], vfull[bh][:], vfull[bh][:], op=ALU.mult)
            nc.vector.tensor_reduce(cl[:], sqf[:].rearrange("p (c o) -> p c o", c=10),
                                    axis=AX.X, op=ALU.add)
            nc.scalar.activation(cl[:], cl[:], ACTF.Sqrt)
            nc.scalar.activation(ecl[bh][:], cl[:], ACTF.Exp)
            nc.tensor.matmul(psZ[:, :10], ones128_sb[:], ecl[bh][:],
                             start=(bh == 0), stop=(bh == 1))
        zrow = rt.tile([1, 16], F32, tag="zrow", name="zrow")
        nc.vector.memset(zrow[:], 0)
        nc.scalar.activation(zrow[:, :10], psZ[:, :10], ACTF.Copy)
        nc.vector.reciprocal(zrow[:, :10], zrow[:, :10])
        nc.sync.dma_start(z_in[:], zrow[:])
        zfull = rt.tile([128, 10], F32, tag="zfull", name="zfull")
        nc.gpsimd.dma_start(zfull[:], mkap(z_in[:], 0, [[0, 128], [1, 10]]))

        tpr = rt.tile([128, 10], F32, tag="tpr", name="tpr")
        tmax = rt.tile([128, 1], F32, tag="tmax", name="tmax")
        mask = rt.tile([128, 10], F32, tag="mask", name="mask")
        flat = rt.tile([128, 160], BF, tag="flat", name="flat")
        flatTa = rt.tile([128, 256], BF, tag="flatTa", name="flatTa")
        flatTb = rt.tile([32, 256], BF, tag="flatTb", name="flatTb")
        h1q = [rt.tile([128, 256], BF, tag=f"h1q{i}", name=f"h1q{i}") for i in range(4)]
        for bh in range(2):
            nc.vector.tensor_tensor(tpr[:], ecl[bh][:], zfull[:], op=ALU.mult)
            nc.vector.tensor_reduce(tmax[:], tpr[:], axis=AX.X, op=ALU.max)
            nc.vector.tensor_scalar(mask[:], tpr[:], tmax[:], None, op0=ALU.is_equal)
            mb = mask[:].unsqueeze(2).broadcast_to([128, 10, 16])
            nc.vector.tensor_tensor(flat[:].rearrange("p (c o) -> p c o", c=10),
                                    vfull[bh][:].rearrange("p (c o) -> p c o", c=10),
                                    mb, op=ALU.mult)
            psT1 = ps_r.tile([128, 128], BF, tag="psT", name="psT1", bufs=2)
            nc.tensor.transpose(psT1[:], flat[:, 0:128], id128_sb[:])
            nc.scalar.activation(flatTa[:, bh * 128:(bh + 1) * 128], psT1[:], ACTF.Copy)
            psT2 = ps_r.tile([32, 128], BF, tag="psT", name="psT2", bufs=2)
            nc.tensor.transpose(psT2[:], flat[:, 128:160], id128_sb[:])
            nc.scalar.activation(flatTb[:, bh * 128:(bh + 1) * 128], psT2[:], ACTF.Copy)
        # fc1: h1 = relu(w1 @ flat + b1)   [512, 256]
        for fc in range(4):
            ps1 = ps_r.tile([128, 256], F32, tag="psD", name="ps1")
            nc.tensor.matmul(ps1[:], w1ta_sb[:, fc * 128:(fc + 1) * 128], flatTa[:],
                             start=True, stop=False)
            nc.tensor.matmul(ps1[:], w1tb_sb[:, fc * 128:(fc + 1) * 128], flatTb[:],
                             start=False, stop=True)
            nc.scalar.activation(h1q[fc][:], ps1[:], ACTF.Relu, bias=b1d_sb[fc][:],
                                 scale=1.0)
        # fc2: h2 = relu(w2 @ h1 + b2), quantized to fp8 * S_H2
        for gc in range(8):
            ps2 = ps_r.tile([128, 256], F32, tag="psD", name="ps2")
            for kc in range(4):
                nc.tensor.matmul(ps2[:], w2t_sb[kc][:, gc * 128:(gc + 1) * 128],
                                 h1q[kc][:], start=(kc == 0), stop=(kc == 3))
            nc.scalar.activation(h2q_sb[:, gc, :], ps2[:], ACTF.Relu,
                                 bias=b2s_sb[gc][:], scale=S_H2)

    # =================== final big layer (fp8 DoubleRow, weights resident) ====
    with tc.tile_pool(name="ps_o", bufs=4, space="PSUM") as ps_o, \
         tc.tile_pool(name="osb", bufs=4) as osbp:
        w3v = w3q_sb[:].rearrange("p (w r j n) -> p w r j n", w=NW, r=4, j=2)
        for w in range(NW):
            for bh in range(2):
                pso = ps_o.tile([128, 512], F32, tag="pso", name="pso")
                for pr in range(4):
                    nc.tensor.matmul(pso[:],
                                     h2q_sb[:, 2 * pr:2 * pr + 2, bh * 128:(bh + 1) * 128],
                                     w3v[:, w, pr], start=(pr == 0), stop=False,
                                     perf_mode=PM.DoubleRow)
                nc.tensor.matmul(pso[:], onesrow_sb[:],
                                 b3q_sb[:, w * 512:(w + 1) * 512],
                                 start=False, stop=True)
                ot = osbp.tile([128, 512], BF, tag="ot", name="ot")
                nc.scalar.activation(ot[:], pso[:], ACTF.Sigmoid, scale=1.0 / (S_H2 * S_W3))
                nc.sync.dma_start(out_ext[bh * 128:(bh + 1) * 128,
                                          w * 512:(w + 1) * 512], ot[:])


_NC_CACHE = {}


def _host_prep(inputs):
    data = np.asarray(inputs["data"], np.float32)
    conv1_w = np.asarray(inputs["conv1_w"], np.float32)
    conv1_b = np.asarray(inputs["conv1_b"], np.float32)
    prim_w = np.asarray(inputs["prim_w"], np.float32)
    prim_b = np.asarray(inputs["prim_b"], np.float32)
    W_digit = np.asarray(inputs["W_digit"], np.float32)
    dec_w1 = np.asarray(inputs["dec_w1"], np.float32)
    dec_b1 = np.asarray(inputs["dec_b1"], np.float32)
    dec_w2 = np.asarray(inputs["dec_w2"], np.float32)
    dec_b2 = np.asarray(inputs["dec_b2"], np.float32)
    dec_w3 = np.asarray(inputs["dec_w3"], np.float32)
    dec_b3 = np.asarray(inputs["dec_b3"], np.float32)

    w1c = np.ascontiguousarray(conv1_w[:, 0].transpose(1, 2, 0).reshape(81, 256)).astype(BF16)
    wp2q = np.ascontiguousarray(
        prim_w.transpose(2, 3, 1, 0).reshape(20736, 256) * S_WP).astype(FP8)
    # Wfull [p=(cl,g,i), (chunk(h,cc), cd, o)]; route r = 256h + 16cc + 2cl + g
    Wv = W_digit.reshape(2, 16, 8, 2, 10, 16, 8)  # [h, cc, cl, g, cd, o, i]
    wfull = np.ascontiguousarray(Wv.transpose(2, 3, 6, 0, 1, 4, 5)).reshape(128, 5120).astype(BF16)
    ones8 = np.zeros((128, 16), np.float32)
    ones8[np.arange(128), np.arange(128) // 8] = 1.0 / 256.0
    w1t = np.ascontiguousarray(dec_w1.T).astype(BF16)
    w2t = np.ascontiguousarray(dec_w2.T).astype(BF16)
    w3t = np.ascontiguousarray(dec_w3.T)  # [1024, 102400]

    common = dict(
        w1c=w1c, b1s=(conv1_b * S_H).reshape(256, 1),
        bp2=prim_b.reshape(256, 1), wp2q=wp2q, wfull=wfull,
        ones8=ones8, id128=np.eye(128, dtype=np.float32).astype(BF16),
        w1t=w1t, b1d=dec_b1.reshape(512, 1),
        w2t=w2t, b2s=(dec_b2 * S_H2).reshape(1024, 1),
        onesrow=np.ones((1, 128), np.float32).astype(BF16),
        ones128=np.ones((128, 1), np.float32),
    )
    in_maps = []
    for c in range(NCORES):
        m = dict(common)
        sw = np.lib.stride_tricks.sliding_window_view(
            data[c * BL:(c + 1) * BL, 0], (9, 9), axis=(1, 2))
        m["pat1h"] = np.ascontiguousarray(
            sw.transpose(3, 4, 1, 2, 0).reshape(81, 4608)).astype(BF16)
        w3c = w3t[:, c * RECL:(c + 1) * RECL] * S_W3   # [1024, 12800]
        m["w3q"] = np.ascontiguousarray(
            w3c.reshape(4, 2, 128, NW, 512).transpose(2, 3, 0, 1, 4).reshape(128, 102400)
        ).astype(FP8)
        m["b3q"] = (dec_b3[c * RECL:(c + 1) * RECL] * (S_H2 * S_W3)).reshape(1, RECL).astype(BF16)
        in_maps.append(m)
    return in_maps


def kernel(**inputs):
    if "nc" not in _NC_CACHE:
        _NC_CACHE["nc"] = build_program()
    nc = _NC_CACHE["nc"]
    in_maps = _host_prep(inputs)
    res = run_bass_kernel_spmd(nc, in_maps, list(range(NCORES)))
    outs = [np.asarray(res.results[c]["out"]).astype(np.float32) for c in range(NCORES)]
    rec = np.concatenate(outs, axis=1)
    return rec.reshape(B, 256, 20, 20)


# revision 6
# speedup vs baseline: 2.2402x; 1.1251x over previous
import sys

sys.path.insert(0, "/opt/trn_rl_repo")

import numpy as np
import ml_dtypes

import concourse.bass as bass
import concourse.mybir as mybir
import concourse.tile as tile
from concourse import bacc
from concourse.bass_utils import run_bass_kernel_spmd

BF16 = ml_dtypes.bfloat16
FP8 = ml_dtypes.float8_e4m3
F32 = mybir.dt.float32
BF = mybir.dt.bfloat16
F8 = mybir.dt.float8e4
ALU = mybir.AluOpType
ACTF = mybir.ActivationFunctionType
AX = mybir.AxisListType
PM = mybir.MatmulPerfMode

NCORES = 8
B = 256
BL = B // NCORES          # 32 local batch
REC = 102400
RECL = REC // NCORES      # 12800 local output cols
NW = RECL // 512          # 25 output windows

S_H = 32.0                # conv1-activation fp8 scale
S_WP = 4096.0             # primary-caps weight fp8 scale
S_H2 = 2.0 ** 23          # h2 fp8 scale
S_W3 = 1024.0             # dec_w3 fp8 scale


def mkap(t, offset, dims):
    """Manual access pattern: dims = [[stride, count], ...] (partition dim first)."""
    return bass.AP(tensor=t.tensor if isinstance(t, bass.AP) else t, offset=offset, ap=dims)


def build_program():
    nc = bacc.Bacc(None, num_devices=NCORES)
    rg = [list(range(NCORES))]

    P = {}
    P["pat1h"] = nc.declare_dram_parameter("pat1h", [81, 4608], BF, isOutput=False)
    P["w1c"] = nc.declare_dram_parameter("w1c", [81, 256], BF, isOutput=False)
    P["b1s"] = nc.declare_dram_parameter("b1s", [256, 1], F32, isOutput=False)
    P["wp2q"] = nc.declare_dram_parameter("wp2q", [20736, 256], F8, isOutput=False)
    P["bp2"] = nc.declare_dram_parameter("bp2", [256, 1], F32, isOutput=False)
    P["wfull"] = nc.declare_dram_parameter("wfull", [128, 5120], BF, isOutput=False)
    P["ones8"] = nc.declare_dram_parameter("ones8", [128, 16], F32, isOutput=False)
    P["id128"] = nc.declare_dram_parameter("id128", [128, 128], BF, isOutput=False)
    P["w1t"] = nc.declare_dram_parameter("w1t", [160, 512], BF, isOutput=False)
    P["b1d"] = nc.declare_dram_parameter("b1d", [512, 1], F32, isOutput=False)
    P["w2t"] = nc.declare_dram_parameter("w2t", [512, 1024], BF, isOutput=False)
    P["b2s"] = nc.declare_dram_parameter("b2s", [1024, 1], F32, isOutput=False)
    P["w3q"] = nc.declare_dram_parameter("w3q", [128, 102400], F8, isOutput=False)
    P["b3q"] = nc.declare_dram_parameter("b3q", [1, RECL], BF, isOutput=False)
    P["onesrow"] = nc.declare_dram_parameter("onesrow", [1, 128], BF, isOutput=False)
    P["ones128"] = nc.declare_dram_parameter("ones128", [128, 1], F32, isOutput=False)
    out_ext = nc.declare_dram_parameter("out", [B, RECL], BF, isOutput=True)

    with tile.TileContext(nc) as tc:
        _body(nc, tc, P, out_ext, rg)
    nc.compile()
    return nc


def _body(nc, tc, P, out_ext, rg):
    es = tc.tile_pool(name="const", bufs=1)
    const = es.__enter__()
    dram_cm = tc.tile_pool(name="dram", bufs=1, space="DRAM")
    dram = dram_cm.__enter__()

    # ---------- DRAM scratch ----------
    warm_in = dram.tile([1, 16], F32, tag="warm_in", name="warm_in")
    warm_out = dram.tile([1, 16], F32, tag="warm_out", name="warm_out")
    xdram = dram.tile([2, 128, 512], BF, tag="xdram", name="xdram")
    warm2_in = dram.tile([16, 320], BF, tag="warm2_in", name="warm2_in")
    warm2_out = dram.tile([16, 320], BF, tag="warm2_out", name="warm2_out")
    ar_in = dram.tile([16, 320], BF, tag="ar_in", name="ar_in")
    ar_out = dram.tile([16, 320], BF, tag="ar_out", name="ar_out")
    c2d = dram.tile([10, 512], BF, tag="c2d", name="c2d")
    z_in = dram.tile([1, 16], F32, tag="z_in", name="z_in")
    z_out = dram.tile([1, 16], F32, tag="z_out", name="z_out")
    vin = dram.tile([BL, 160], F32, tag="vin", name="vin")
    vall = dram.tile([NCORES, BL, 160], F32, tag="vall", name="vall")

    # ---------- warmup collective (absorb first-collective setup cost) ----------
    zw = const.tile([1, 16], F32, tag="zw", name="zw")
    nc.vector.memset(zw[:], 0)
    nc.sync.dma_start(warm_in[:], zw[:])
    nc.gpsimd.collective_compute(
        "AllReduce", ALU.add, replica_groups=rg,
        ins=[warm_in[:].opt()], outs=[warm_out[:].opt()])
    zw2 = const.tile([16, 320], BF, tag="zw2", name="zw2")
    nc.vector.memset(zw2[:], 0)
    nc.sync.dma_start(warm2_in[:], zw2[:])
    nc.gpsimd.collective_compute(
        "AllReduce", ALU.add, replica_groups=rg,
        ins=[warm2_in[:].opt()], outs=[warm2_out[:].opt()])

    # ---------- constants to SBUF ----------
    w1c_sb = const.tile([81, 256], BF, tag="w1c", name="w1c")
    nc.sync.dma_start(w1c_sb[:], P["w1c"][:])
    b1s_sb = [const.tile([128, 1], F32, tag=f"b1s{h}", name=f"b1s{h}") for h in range(2)]
    bp2_sb = [const.tile([128, 1], F32, tag=f"bp2{h}", name=f"bp2{h}") for h in range(2)]
    for h in range(2):
        nc.sync.dma_start(b1s_sb[h][:], P["b1s"][h * 128:(h + 1) * 128, :])
        nc.sync.dma_start(bp2_sb[h][:], P["bp2"][h * 128:(h + 1) * 128, :])
    wfull_sb = const.tile([128, 5120], BF, tag="wfull", name="wfull")
    nc.scalar.dma_start(wfull_sb[:], P["wfull"][:])
    ones8_sb = const.tile([128, 16], F32, tag="ones8", name="ones8")
    nc.sync.dma_start(ones8_sb[:], P["ones8"][:])
    id128_sb = const.tile([128, 128], BF, tag="id128", name="id128")
    nc.sync.dma_start(id128_sb[:], P["id128"][:])
    w1ta_sb = const.tile([128, 512], BF, tag="w1ta", name="w1ta")
    nc.sync.dma_start(w1ta_sb[:], P["w1t"][0:128, :])
    w1tb_sb = const.tile([32, 512], BF, tag="w1tb", name="w1tb")
    nc.sync.dma_start(w1tb_sb[:], P["w1t"][128:160, :])
    b1d_sb = [const.tile([128, 1], F32, tag=f"b1d{i}", name=f"b1d{i}") for i in range(4)]
    for i in range(4):
        nc.sync.dma_start(b1d_sb[i][:], P["b1d"][i * 128:(i + 1) * 128, :])
    w2t_sb = [const.tile([128, 1024], BF, tag=f"w2t{i}", name=f"w2t{i}") for i in range(4)]
    for i in range(4):
        nc.scalar.dma_start(w2t_sb[i][:], P["w2t"][i * 128:(i + 1) * 128, :])
    b2s_sb = [const.tile([128, 1], F32, tag=f"b2s{i}", name=f"b2s{i}") for i in range(8)]
    for i in range(8):
        nc.sync.dma_start(b2s_sb[i][:], P["b2s"][i * 128:(i + 1) * 128, :])
    onesrow_sb = const.tile([1, 128], BF, tag="onesrow", name="onesrow")
    nc.sync.dma_start(onesrow_sb[:], P["onesrow"][:])
    ones128_sb = const.tile([128, 1], F32, tag="ones128", name="ones128")
    nc.sync.dma_start(ones128_sb[:], P["ones128"][:])
    b3q_sb = const.tile([1, RECL], BF, tag="b3q", name="b3q")
    nc.scalar.dma_start(b3q_sb[:], P["b3q"][:])
    # big fp8 decoder weight: prefetch fully into SBUF (overlaps front+routing)
    w3q_sb = const.tile([128, 102400], F8, tag="w3q", name="w3q")
    nc.scalar.dma_start(w3q_sb[:], P["w3q"][:])

    # persistent mid-size tiles
    xT_sb = const.tile([128, 1024], BF, tag="xT", name="xT")      # [(cl,g,i), (chunk,b)]
    x2_sb = const.tile([BL, 4096], BF, tag="x2", name="x2")       # [b, (chunk,cl,g,i)]
    wc_sb = const.tile([128, 5120], BF, tag="wc", name="wc")      # c-weighted W / prod scratch
    h2q_sb = const.tile([128, 8, 256], F8, tag="h2q", name="h2q")  # [feat%128, kc, b]

    # =================== conv1 + primary caps (fp8 DoubleRow) ===================
    with tc.tile_pool(name="front", bufs=1) as front, \
         tc.tile_pool(name="wp2p", bufs=4) as wp2p, \
         tc.tile_pool(name="ps_f", bufs=2, space="PSUM") as ps_f:
        pat1 = front.tile([81, 4608], BF, tag="pat1", name="pat1")
        nc.sync.dma_start(pat1[:], P["pat1h"][:])
        H = front.tile([128, 2, 4608], F8, tag="H", name="H")
        for h in range(2):
            for w in range(9):
                ps = ps_f.tile([128, 512], F32, tag="c1ps", name="c1ps")
                nc.tensor.matmul(ps[:], w1c_sb[:, h * 128:(h + 1) * 128],
                                 pat1[:, w * 512:(w + 1) * 512],
                                 start=True, stop=True)
                nc.scalar.activation(H[:, h, w * 512:(w + 1) * 512], ps[:],
                                     ACTF.Relu, bias=b1s_sb[h][:], scale=S_H)
        U = [front.tile([128, 512], F32, tag=f"U{h}", name=f"U{h}") for h in range(2)]
        psU = [ps_f.tile([128, 512], F32, tag=f"Ups{h}", name=f"Ups{h}", bufs=1) for h in range(2)]
        Hv = H[:].rearrange("p c (y x b) -> p c y x b", y=12, x=12)
        for u in range(81):
            dy, dx = divmod(u, 9)
            wt = wp2p.tile([128, 2, 256], F8, tag="wp2t", name="wp2t")
            nc.sync.dma_start(wt[:], mkap(P["wp2q"], u * 65536,
                                          [[256, 128], [32768, 2], [1, 256]]))
            rhs = Hv[:, :, dy:dy + 4, dx:dx + 4, :]
            for h in range(2):
                nc.tensor.matmul(psU[h][:], wt[:, :, h * 128:(h + 1) * 128], rhs,
                                 start=(u == 0), stop=(u == 80),
                                 perf_mode=PM.DoubleRow)
        for h in range(2):
            nc.scalar.activation(U[h][:], psU[h][:], ACTF.Identity,
                                 bias=bp2_sb[h][:], scale=1.0 / (S_H * S_WP))

        # ---- squash -> x (bf16), to DRAM, reload transposed ----
        usq = front.tile([128, 512], F32, tag="usq", name="usq")
        sn = front.tile([128, 64], F32, tag="sn", name="sn")
        g = front.tile([128, 64], F32, tag="g", name="g")
        gt = front.tile([128, 64], F32, tag="gt", name="gt")
        X = front.tile([128, 512], BF, tag="X", name="X")
        for h in range(2):
            nc.vector.tensor_tensor(usq[:], U[h][:], U[h][:], op=ALU.mult)
            uview = usq[:].rearrange("p (g i b) -> p g b i", g=2, i=8)
            nc.vector.tensor_reduce(sn[:].rearrange("p (g b) -> p g b", g=2),
                                    uview, axis=AX.X, op=ALU.add)
            nc.scalar.activation(gt[:], sn[:], ACTF.Sqrt)
            nc.vector.tensor_scalar_add(g[:], sn[:], 1.0)
            nc.vector.reciprocal(g[:], g[:])
            nc.vector.tensor_tensor(g[:], g[:], gt[:], op=ALU.mult)
            gb = g[:].rearrange("p (g b) -> p g b", g=2).unsqueeze(2).broadcast_to(
                [128, 2, 8, BL])
            nc.vector.tensor_tensor(X[:].rearrange("p (g i b) -> p g i b", g=2, i=8),
                                    U[h][:].rearrange("p (g i b) -> p g i b", g=2, i=8),
                                    gb, op=ALU.mult)
            nc.sync.dma_start(xdram[h], X[:])
        xsrc = mkap(xdram[:], 0, [[32, 128], [4096, 32], [1, 32]])
        nc.sync.dma_start(xT_sb[:], xsrc)

    # =================== routing (3 iters, matmul-factored) ===================
    with tc.tile_pool(name="route", bufs=1) as rt, \
         tc.tile_pool(name="ps_r", bufs=1, space="PSUM") as ps_r:
        s_sb = rt.tile([BL, 160], F32, tag="s_sb", name="s_sb")
        sq = rt.tile([BL, 160], F32, tag="sq", name="sq")
        num = rt.tile([BL, 160], F32, tag="num", name="num")
        dn = rt.tile([BL, 160], F32, tag="dn", name="dn")
        v_sb = rt.tile([BL, 160], F32, tag="v_sb", name="v_sb")
        vq = rt.tile([BL, 160], BF, tag="vq", name="vq")
        t1 = rt.tile([128, 320], F32, tag="t1", name="t1")
        braw = rt.tile([16, 320], BF, tag="braw", name="braw")
        b_acc = rt.tile([16, 320], BF, tag="b_acc", name="b_acc")
        csf = rt.tile([10, 512], BF, tag="csf", name="csf")
        rmax = rt.tile([10, 1], F32, tag="rmax", name="rmax")
        nbias = rt.tile([10, 1], F32, tag="nbias", name="nbias")
        esum = rt.tile([10, 1], F32, tag="esum", name="esum")
        c_sb = rt.tile([10, 512], F32, tag="c_sb", name="c_sb")
        c_bf = rt.tile([10, 512], BF, tag="c_bf", name="c_bf")
        crep2 = rt.tile([128, 320], BF, tag="crep2", name="crep2")

        for it in range(3):
            # ---- s = sum_r c_r u_hat  via matmul over (r,i) ----
            psS = ps_r.tile([BL, 160], F32, tag="psS", name="psS")
            if it == 0:
                for j in range(32):
                    nc.tensor.matmul(psS[:], xT_sb[:, j * 32:(j + 1) * 32],
                                     wfull_sb[:, j * 160:(j + 1) * 160],
                                     start=(j == 0), stop=(j == 31))
                nc.vector.tensor_scalar(s_sb[:], psS[:], 1.0 / 512.0, None, op0=ALU.mult)
            else:
                cvv = crep2[:].rearrange("p (c m) -> p m c", c=10).unsqueeze(3)
                wfv = wfull_sb[:].rearrange("p (m c o) -> p m c o", m=32, c=10)
                wcv = wc_sb[:].rearrange("p (m c o) -> p m c o", m=32, c=10)
                for seg in range(4):
                    sl = slice(8 * seg, 8 * (seg + 1))
                    nc.vector.tensor_tensor(
                        wcv[:, sl], wfv[:, sl],
                        cvv[:, sl].broadcast_to([128, 8, 10, 16]), op=ALU.mult)
                    for j in range(8 * seg, 8 * (seg + 1)):
                        nc.tensor.matmul(psS[:], xT_sb[:, j * 32:(j + 1) * 32],
                                         wc_sb[:, j * 160:(j + 1) * 160],
                                         start=(j == 0), stop=(j == 31))
                nc.vector.tensor_copy(s_sb[:], psS[:])
            # ---- elementwise squash: v = sq*s/((1+sq)*sqrt(sq)) ----
            nc.vector.tensor_tensor(sq[:], s_sb[:], s_sb[:], op=ALU.mult)
            nc.vector.tensor_tensor(num[:], sq[:], s_sb[:], op=ALU.mult)
            nc.vector.tensor_scalar_add(dn[:], sq[:], 1.0)
            nc.scalar.activation(sq[:], sq[:], ACTF.Sqrt)
            nc.vector.tensor_tensor(dn[:], dn[:], sq[:], op=ALU.mult)
            nc.vector.reciprocal(dn[:], dn[:])
            nc.vector.tensor_tensor(v_sb[:], num[:], dn[:], op=ALU.mult)

            if it == 2:
                break
            if it == 0:
                # x2 via 32 tensor transposes of xT chunks (overlaps squash)
                for j in range(32):
                    psT = ps_r.tile([32, 128], BF, tag="psT", name="psT", bufs=2)
                    nc.tensor.transpose(psT[:], xT_sb[:, j * 32:(j + 1) * 32], id128_sb[:])
                    nc.vector.tensor_copy(x2_sb[:, j * 128:(j + 1) * 128], psT[:])
            # ---- G[(r,i),(cd,o)] = sum_b x v  (32 matmuls) ----
            nc.vector.tensor_copy(vq[:], v_sb[:])
            for j in range(32):
                psG = ps_r.tile([128, 160], F32, tag="psG", name="psG", bufs=2)
                nc.tensor.matmul(psG[:], x2_sb[:, j * 128:(j + 1) * 128], vq[:],
                                 start=True, stop=True)
                nc.vector.tensor_tensor(wc_sb[:, j * 160:(j + 1) * 160], psG[:],
                                        wfull_sb[:, j * 160:(j + 1) * 160], op=ALU.mult)
            # ---- a_mean = sum_{o,i} W . G ----
            nc.vector.tensor_reduce(
                t1[:].rearrange("p (m c) -> p m c", m=32),
                wc_sb[:].rearrange("p (m c o) -> p m c o", m=32, c=10),
                axis=AX.X, op=ALU.add)
            psA = ps_r.tile([16, 320], F32, tag="psA", name="psA")
            nc.tensor.matmul(psA[:], ones8_sb[:],
                             t1[:].rearrange("p (m c) -> p c m", m=32),
                             start=True, stop=True)
            # fold previous b-state into the reduce: AR output IS the new b
            if it == 0:
                nc.vector.tensor_copy(braw[:], psA[:])
            else:
                nc.vector.scalar_tensor_tensor(braw[:], b_acc[:], 0.125, psA[:],
                                               op0=ALU.mult, op1=ALU.add)
            nc.sync.dma_start(ar_in[:], braw[:])
            nc.gpsimd.collective_compute(
                "AllReduce", ALU.add, replica_groups=rg,
                ins=[ar_in[:].opt()], outs=[ar_out[:].opt()])
            # ---- softmax over routes: b [16,(cd,m)] -> csf [10,(q,m)] ----
            nc.sync.dma_start(csf[:], mkap(ar_out[:], 0, [[32, 10], [320, 16], [1, 32]]))
            nc.gpsimd.dma_start(b_acc[:], ar_out[:])
            nc.vector.tensor_reduce(rmax[:], csf[:], axis=AX.X, op=ALU.max)
            nc.scalar.mul(nbias[:], rmax[:], -1.0)
            nc.scalar.activation(c_sb[:], csf[:], ACTF.Exp, bias=nbias[:], scale=1.0)
            nc.vector.tensor_reduce(esum[:], c_sb[:], axis=AX.X, op=ALU.add)
            nc.vector.reciprocal(esum[:], esum[:])
            nc.vector.tensor_scalar_mul(c_sb[:], c_sb[:], esum[:])
            nc.vector.tensor_copy(c_bf[:], c_sb[:])
            nc.sync.dma_start(c2d[:], c_bf[:])
            for q in range(16):
                src = mkap(c2d[:], q * 32, [[0, 8], [512, 10], [1, 32]])
                eng = nc.sync if q % 2 == 0 else nc.gpsimd
                eng.dma_start(crep2[8 * q:8 * (q + 1), :], src)

        # =================== tail: AllGather v, full-batch decoder ===========
        nc.sync.dma_start(vin[:], v_sb[:])
        nc.gpsimd.collective_compute(
            "AllGather", ALU.bypass, replica_groups=rg,
            ins=[vin[:].opt()], outs=[vall[:].opt()])

        vfull = [rt.tile([128, 160], F32, tag=f"vf{bh}", name=f"vf{bh}") for bh in range(2)]
        ecl = [rt.tile([128, 10], F32, tag=f"ecl{bh}", name=f"ecl{bh}") for bh in range(2)]
        sqf = rt.tile([128, 160], F32, tag="sqf", name="sqf")
        cl = rt.tile([128, 10], F32, tag="cl", name="cl")
        psZ = ps_r.tile([1, 16], F32, tag="psA", name="psZ", bufs=1)
        for bh in range(2):
            nc.sync.dma_start(vfull[bh][:],
                              mkap(vall[:], bh * 128 * 160, [[160, 128], [1, 160]]))
            nc.vector.tensor_tensor(sqf[:], vfull[bh][:], vfull[bh][:], op=ALU.mult)
            nc.vector.tensor_reduce(cl[:], sqf[:].rearrange("p (c o) -> p c o", c=10),
                                    axis=AX.X, op=ALU.add)
            nc.scalar.activation(cl[:], cl[:], ACTF.Sqrt)
            nc.scalar.activation(ecl[bh][:], cl[:], ACTF.Exp)
            nc.tensor.matmul(psZ[:, :10], ones128_sb[:], ecl[bh][:],
                             start=(bh == 0), stop=(bh == 1))
        zrow = rt.tile([1, 16], F32, tag="zrow", name="zrow")
        nc.vector.memset(zrow[:], 0)
        nc.vector.tensor_copy(zrow[:, :10], psZ[:, :10])
        nc.vector.reciprocal(zrow[:, :10], zrow[:, :10])
        nc.sync.dma_start(z_in[:], zrow[:])
        zfull = rt.tile([128, 10], F32, tag="zfull", name="zfull")
        nc.gpsimd.dma_start(zfull[:], mkap(z_in[:], 0, [[0, 128], [1, 10]]))

        tpr = rt.tile([128, 10], F32, tag="tpr", name="tpr")
        tmax = rt.tile([128, 1], F32, tag="tmax", name="tmax")
        mask = rt.tile([128, 10], F32, tag="mask", name="mask")
        flat = rt.tile([128, 160], BF, tag="flat", name="flat")
        flatTa = rt.tile([128, 256], BF, tag="flatTa", name="flatTa")
        flatTb = rt.tile([32, 256], BF, tag="flatTb", name="flatTb")
        h1q = [rt.tile([128, 256], BF, tag=f"h1q{i}", name=f"h1q{i}") for i in range(4)]
        for bh in range(2):
            nc.vector.tensor_tensor(tpr[:], ecl[bh][:], zfull[:], op=ALU.mult)
            nc.vector.tensor_reduce(tmax[:], tpr[:], axis=AX.X, op=ALU.max)
            nc.vector.tensor_scalar(mask[:], tpr[:], tmax[:], None, op0=ALU.is_equal)
            mb = mask[:].unsqueeze(2).broadcast_to([128, 10, 16])
            nc.vector.tensor_tensor(flat[:].rearrange("p (c o) -> p c o", c=10),
                                    vfull[bh][:].rearrange("p (c o) -> p c o", c=10),
                                    mb, op=ALU.mult)
            psT1 = ps_r.tile([128, 128], BF, tag="psT", name="psT1", bufs=2)
            nc.tensor.transpose(psT1[:], flat[:, 0:128], id128_sb[:])
            nc.vector.tensor_copy(flatTa[:, bh * 128:(bh + 1) * 128], psT1[:])
            psT2 = ps_r.tile([32, 128], BF, tag="psT", name="psT2", bufs=2)
            nc.tensor.transpose(psT2[:], flat[:, 128:160], id128_sb[:])
            nc.vector.tensor_copy(flatTb[:, bh * 128:(bh + 1) * 128], psT2[:])
        # fc1: h1 = relu(w1 @ flat + b1)   [512, 256]
        for fc in range(4):
            ps1 = ps_r.tile([128, 256], F32, tag="psD", name="ps1", bufs=2)
            nc.tensor.matmul(ps1[:], w1ta_sb[:, fc * 128:(fc + 1) * 128], flatTa[:],
                             start=True, stop=False)
            nc.tensor.matmul(ps1[:], w1tb_sb[:, fc * 128:(fc + 1) * 128], flatTb[:],
                             start=False, stop=True)
            nc.scalar.activation(h1q[fc][:], ps1[:], ACTF.Relu, bias=b1d_sb[fc][:],
                                 scale=1.0)
        # fc2: h2 = relu(w2 @ h1 + b2), quantized to fp8 * S_H2
        for gc in range(8):
            ps2 = ps_r.tile([128, 256], F32, tag="psD", name="ps2", bufs=2)
            for kc in range(4):
                nc.tensor.matmul(ps2[:], w2t_sb[kc][:, gc * 128:(gc + 1) * 128],
                                 h1q[kc][:], start=(kc == 0), stop=(kc == 3))
            nc.scalar.activation(h2q_sb[:, gc, :], ps2[:], ACTF.Relu,
                                 bias=b2s_sb[gc][:], scale=S_H2)

    # =================== final big layer (fp8 DoubleRow, weights resident) ====
    with tc.tile_pool(name="ps_o", bufs=4, space="PSUM") as ps_o, \
         tc.tile_pool(name="osb", bufs=4) as osbp:
        w3v = w3q_sb[:].rearrange("p (w r j n) -> p w r j n", w=NW, r=4, j=2)
        for w in range(NW):
            for bh in range(2):
                pso = ps_o.tile([128, 512], F32, tag="pso", name="pso")
                for pr in range(4):
                    nc.tensor.matmul(pso[:],
                                     h2q_sb[:, 2 * pr:2 * pr + 2, bh * 128:(bh + 1) * 128],
                                     w3v[:, w, pr], start=(pr == 0), stop=False,
                                     perf_mode=PM.DoubleRow)
                nc.tensor.matmul(pso[:], onesrow_sb[:],
                                 b3q_sb[:, w * 512:(w + 1) * 512],
                                 start=False, stop=True)
                ot = osbp.tile([128, 512], BF, tag="ot", name="ot")
                nc.scalar.activation(ot[:], pso[:], ACTF.Sigmoid, scale=1.0 / (S_H2 * S_W3))
                nc.sync.dma_start(out_ext[bh * 128:(bh + 1) * 128,
                                          w * 512:(w + 1) * 512], ot[:])


_NC_CACHE = {}


def _host_prep(inputs):
    data = np.asarray(inputs["data"], np.float32)
    conv1_w = np.asarray(inputs["conv1_w"], np.float32)
    conv1_b = np.asarray(inputs["conv1_b"], np.float32)
    prim_w = np.asarray(inputs["prim_w"], np.float32)
    prim_b = np.asarray(inputs["prim_b"], np.float32)
    W_digit = np.asarray(inputs["W_digit"], np.float32)
    dec_w1 = np.asarray(inputs["dec_w1"], np.float32)
    dec_b1 = np.asarray(inputs["dec_b1"], np.float32)
    dec_w2 = np.asarray(inputs["dec_w2"], np.float32)
    dec_b2 = np.asarray(inputs["dec_b2"], np.float32)
    dec_w3 = np.asarray(inputs["dec_w3"], np.float32)
    dec_b3 = np.asarray(inputs["dec_b3"], np.float32)

    w1c = np.ascontiguousarray(conv1_w[:, 0].transpose(1, 2, 0).reshape(81, 256)).astype(BF16)
    wp2q = np.ascontiguousarray(
        prim_w.transpose(2, 3, 1, 0).reshape(20736, 256) * S_WP).astype(FP8)
    # Wfull [p=(cl,g,i), (chunk(h,cc), cd, o)]; route r = 256h + 16cc + 2cl + g
    Wv = W_digit.reshape(2, 16, 8, 2, 10, 16, 8)  # [h, cc, cl, g, cd, o, i]
    wfull = np.ascontiguousarray(Wv.transpose(2, 3, 6, 0, 1, 4, 5)).reshape(128, 5120).astype(BF16)
    ones8 = np.zeros((128, 16), np.float32)
    ones8[np.arange(128), np.arange(128) // 8] = 1.0 / 256.0
    w1t = np.ascontiguousarray(dec_w1.T).astype(BF16)
    w2t = np.ascontiguousarray(dec_w2.T).astype(BF16)
    w3t = np.ascontiguousarray(dec_w3.T)  # [1024, 102400]

    common = dict(
        w1c=w1c, b1s=(conv1_b * S_H).reshape(256, 1),
        bp2=prim_b.reshape(256, 1), wp2q=wp2q, wfull=wfull,
        ones8=ones8, id128=np.eye(128, dtype=np.float32).astype(BF16),
        w1t=w1t, b1d=dec_b1.reshape(512, 1),
        w2t=w2t, b2s=(dec_b2 * S_H2).reshape(1024, 1),
        onesrow=np.ones((1, 128), np.float32).astype(BF16),
        ones128=np.ones((128, 1), np.float32),
    )
    in_maps = []
    for c in range(NCORES):
        m = dict(common)
        sw = np.lib.stride_tricks.sliding_window_view(
            data[c * BL:(c + 1) * BL, 0], (9, 9), axis=(1, 2))
        m["pat1h"] = np.ascontiguousarray(
            sw.transpose(3, 4, 1, 2, 0).reshape(81, 4608)).astype(BF16)
        w3c = w3t[:, c * RECL:(c + 1) * RECL] * S_W3   # [1024, 12800]
        m["w3q"] = np.ascontiguousarray(
            w3c.reshape(4, 2, 128, NW, 512).transpose(2, 3, 0, 1, 4).reshape(128, 102400)
        ).astype(FP8)
        m["b3q"] = (dec_b3[c * RECL:(c + 1) * RECL] * (S_H2 * S_W3)).reshape(1, RECL).astype(BF16)
        in_maps.append(m)
    return in_maps


def kernel(**inputs):
    if "nc" not in _NC_CACHE:
        _NC_CACHE["nc"] = build_program()
    nc = _NC_CACHE["nc"]
    in_maps = _host_prep(inputs)
    res = run_bass_kernel_spmd(nc, in_maps, list(range(NCORES)))
    outs = [np.asarray(res.results[c]["out"]).astype(np.float32) for c in range(NCORES)]
    rec = np.concatenate(outs, axis=1)
    return rec.reshape(B, 256, 20, 20)
